# revision 5
# baseline (speedup 1.0000x reference)
"""CenterLoss kernel for Trainium2, data-parallel over batch across 8 cores.

loss = mean_b clip(||x_b - centers[labels_b]||^2, 1e-12, 1e12)

Per core: 256 rows of x/labels (batch shard), full replicated centers table.
On device: indirect-DMA gather of the 256 matching center rows, fused
(x-c)^2 row-sum on the vector engine, per-row clip, cross-partition sum via
a [128,1]x[128,1] matmul against ones. Host sums the 8 per-core partials
and divides by the global batch.
"""

import numpy as np

NUM_CLASSES = 100000
FEAT_DIM = 256
BATCH = 2048
N_CORES = 8
P = 128
B_LOCAL = BATCH // N_CORES          # 256 rows per core
N_TILES = B_LOCAL // P              # 2 tiles of 128 rows

_CACHE: dict = {}


def _build_bass():
    import concourse.bass as bass
    import concourse.bacc as bacc
    import concourse.mybir as mybir
    from concourse.tile import TileContext

    f32 = mybir.dt.float32
    i32 = mybir.dt.int32

    nc = bacc.Bacc(None, target_bir_lowering=False)
    x_in = nc.dram_tensor("x", [B_LOCAL, FEAT_DIM], f32, kind="ExternalInput")
    lab_in = nc.dram_tensor("labels", [B_LOCAL, 1], i32, kind="ExternalInput")
    cen_in = nc.dram_tensor(
        "centers", [NUM_CLASSES, FEAT_DIM], f32, kind="ExternalInput"
    )
    out = nc.dram_tensor("partial", [1, 1], f32, kind="ExternalOutput")

    # View the 256 local rows as [128 partitions, 2 tiles, 256 feat]:
    # element [p, t, d] = row t*128+p.
    x_v = x_in.rearrange("(t p) d -> p t d", p=P)
    lab_v = lab_in.rearrange("(t p) 1 -> p t", p=P)

    with TileContext(nc) as tc:
        with (
            tc.tile_pool(name="sbuf", bufs=1) as pool,
            tc.tile_pool(name="psum", bufs=1, space="PSUM") as psum_pool,
        ):
            ones = pool.tile([P, 1], f32)
            nc.vector.memset(ones[:], 1.0)

            lab_t = pool.tile([P, N_TILES], i32)
            nc.sync.dma_start(out=lab_t[:], in_=lab_v)

            x_t = pool.tile([P, N_TILES, FEAT_DIM], f32)
            nc.sync.dma_start(out=x_t[:], in_=x_v)

            c_t = pool.tile([P, N_TILES, FEAT_DIM], f32)
            for t in range(N_TILES):
                nc.gpsimd.indirect_dma_start(
                    out=c_t[:, t, :],
                    out_offset=None,
                    in_=cen_in[:],
                    in_offset=bass.IndirectOffsetOnAxis(
                        ap=lab_t[:, t : t + 1], axis=0
                    ),
                )

            diff = pool.tile([P, N_TILES, FEAT_DIM], f32)
            nc.vector.tensor_sub(diff[:], x_t[:], c_t[:])
            sq = pool.tile([P, N_TILES, FEAT_DIM], f32)
            nc.vector.tensor_mul(sq[:], diff[:], diff[:])
            dist = pool.tile([P, N_TILES], f32)
            nc.vector.reduce_sum(
                out=dist[:], in_=sq[:], axis=mybir.AxisListType.X
            )

            dist_c = pool.tile([P, N_TILES], f32)
            nc.vector.tensor_scalar(
                out=dist_c[:],
                in0=dist[:],
                scalar1=1e-12,
                scalar2=1e12,
                op0=mybir.AluOpType.max,
                op1=mybir.AluOpType.min,
            )

            rowsum = pool.tile([P, 1], f32)
            nc.vector.reduce_sum(
                out=rowsum[:], in_=dist_c[:], axis=mybir.AxisListType.X
            )

            ps = psum_pool.tile([1, 1], f32, space="PSUM")
            nc.tensor.matmul(
                out=ps[:], lhsT=rowsum[:], rhs=ones[:], start=True, stop=True
            )

            res = pool.tile([1, 1], f32)
            nc.vector.tensor_copy(res[:], ps[:])
            nc.sync.dma_start(out=out[:, :], in_=res[:])

    nc.finalize()
    return nc


def _get_bass():
    if "nc" not in _CACHE:
        _CACHE["nc"] = _build_bass()
    return _CACHE["nc"]


def _run(in_maps, trace=False, **kwargs):
    from concourse.bass_utils import run_bass_kernel_spmd

    nc = _get_bass()
    return run_bass_kernel_spmd(
        nc, in_maps, core_ids=list(range(N_CORES)), trace=trace, **kwargs
    )


def _make_in_maps(x, labels, centers):
    x = np.ascontiguousarray(np.asarray(x), dtype=np.float32)
    labels = np.asarray(labels).astype(np.int32).reshape(BATCH, 1)
    centers = np.ascontiguousarray(np.asarray(centers), dtype=np.float32)
    in_maps = []
    for c in range(N_CORES):
        lo, hi = c * B_LOCAL, (c + 1) * B_LOCAL
        in_maps.append(
            {
                "x": np.ascontiguousarray(x[lo:hi]),
                "labels": np.ascontiguousarray(labels[lo:hi]),
                "centers": centers,
            }
        )
    return in_maps


def kernel(x, labels, centers):
    in_maps = _make_in_maps(x, labels, centers)
    res = _run(in_maps)
    total = sum(float(r["partial"][0, 0]) for r in res.results)
    return np.float32(total / BATCH)


# revision 6
# speedup vs baseline: 1.0055x; 1.0055x over previous
"""CenterLoss kernel for Trainium2, data-parallel over batch across 8 cores.

loss = mean_b clip(||x_b - centers[labels_b]||^2, 1e-12, 1e12)

Per core: 256 rows of x/labels (batch shard), full replicated centers table.
On device: indirect-DMA gather of the 256 matching center rows, fused
(x-c)^2 row-sum on the vector engine, per-row clip, cross-partition sum via
a [128,1]x[128,1] matmul against ones. Host sums the 8 per-core partials
and divides by the global batch.
"""

import numpy as np

NUM_CLASSES = 100000
FEAT_DIM = 256
BATCH = 2048
N_CORES = 8
P = 128
B_LOCAL = BATCH // N_CORES          # 256 rows per core
N_TILES = B_LOCAL // P              # 2 tiles of 128 rows

_CACHE: dict = {}


def _build_bass():
    import concourse.bass as bass
    import concourse.bacc as bacc
    import concourse.mybir as mybir
    from concourse.tile import TileContext

    f32 = mybir.dt.float32
    i32 = mybir.dt.int32

    nc = bacc.Bacc(None, target_bir_lowering=False)
    x_in = nc.dram_tensor("x", [B_LOCAL, FEAT_DIM], f32, kind="ExternalInput")
    lab_in = nc.dram_tensor("labels", [B_LOCAL, 1], i32, kind="ExternalInput")
    cen_in = nc.dram_tensor(
        "centers", [NUM_CLASSES, FEAT_DIM], f32, kind="ExternalInput"
    )
    out = nc.dram_tensor("partial", [1, 1], f32, kind="ExternalOutput")

    # View the 256 local rows as [128 partitions, 2 tiles, 256 feat]:
    # element [p, t, d] = row t*128+p.
    x_v = x_in.rearrange("(t p) d -> p t d", p=P)
    lab_v = lab_in.rearrange("(t p) 1 -> p t", p=P)

    with TileContext(nc) as tc:
        with (
            tc.tile_pool(name="sbuf", bufs=1) as pool,
            tc.tile_pool(name="psum", bufs=1, space="PSUM") as psum_pool,
        ):
            ones = pool.tile([P, 1], f32)
            nc.vector.memset(ones[:], 1.0)

            lab_t = pool.tile([P, N_TILES], i32)
            nc.gpsimd.dma_start(out=lab_t[:], in_=lab_v)

            x_t = pool.tile([P, N_TILES, FEAT_DIM], f32)
            nc.sync.dma_start(out=x_t[:], in_=x_v)

            c_t = pool.tile([P, N_TILES, FEAT_DIM], f32)
            diff = pool.tile([P, N_TILES, FEAT_DIM], f32)
            sq = pool.tile([P, N_TILES, FEAT_DIM], f32)
            dist = pool.tile([P, N_TILES], f32)
            for t in range(N_TILES):
                nc.gpsimd.indirect_dma_start(
                    out=c_t[:, t, :],
                    out_offset=None,
                    in_=cen_in[:],
                    in_offset=bass.IndirectOffsetOnAxis(
                        ap=lab_t[:, t : t + 1], axis=0
                    ),
                )
            for t in range(N_TILES):
                nc.vector.tensor_sub(
                    diff[:, t, :], x_t[:, t, :], c_t[:, t, :]
                )
                nc.scalar.activation(
                    out=sq[:, t, :],
                    in_=diff[:, t, :],
                    func=mybir.ActivationFunctionType.Square,
                    accum_out=dist[:, t : t + 1],
                )

            dist_c = pool.tile([P, N_TILES], f32)
            nc.vector.tensor_scalar(
                out=dist_c[:],
                in0=dist[:],
                scalar1=1e-12,
                scalar2=1e12,
                op0=mybir.AluOpType.max,
                op1=mybir.AluOpType.min,
            )

            rowsum = pool.tile([P, 1], f32)
            nc.vector.reduce_sum(
                out=rowsum[:], in_=dist_c[:], axis=mybir.AxisListType.X
            )

            ps = psum_pool.tile([1, 1], f32, space="PSUM")
            nc.tensor.matmul(
                out=ps[:], lhsT=rowsum[:], rhs=ones[:], start=True, stop=True
            )

            res = pool.tile([1, 1], f32)
            nc.vector.tensor_copy(res[:], ps[:])
            nc.sync.dma_start(out=out[:, :], in_=res[:])

    nc.finalize()
    return nc


def _get_bass():
    if "nc" not in _CACHE:
        _CACHE["nc"] = _build_bass()
    return _CACHE["nc"]


def _run(in_maps, trace=False, **kwargs):
    from concourse.bass_utils import run_bass_kernel_spmd

    nc = _get_bass()
    return run_bass_kernel_spmd(
        nc, in_maps, core_ids=list(range(N_CORES)), trace=trace, **kwargs
    )


def _make_in_maps(x, labels, centers):
    x = np.ascontiguousarray(np.asarray(x), dtype=np.float32)
    labels = np.asarray(labels).astype(np.int32).reshape(BATCH, 1)
    centers = np.ascontiguousarray(np.asarray(centers), dtype=np.float32)
    in_maps = []
    for c in range(N_CORES):
        lo, hi = c * B_LOCAL, (c + 1) * B_LOCAL
        in_maps.append(
            {
                "x": np.ascontiguousarray(x[lo:hi]),
                "labels": np.ascontiguousarray(labels[lo:hi]),
                "centers": centers,
            }
        )
    return in_maps


def kernel(x, labels, centers):
    in_maps = _make_in_maps(x, labels, centers)
    res = _run(in_maps)
    total = sum(float(r["partial"][0, 0]) for r in res.results)
    return np.float32(total / BATCH)


# revision 7
# speedup vs baseline: 1.0572x; 1.0513x over previous
"""CenterLoss kernel for Trainium2, data-parallel over batch across 8 cores.

loss = mean_b clip(||x_b - centers[labels_b]||^2, 1e-12, 1e12)

Per core: 256 rows of x/labels (batch shard), full replicated centers table.
On device: indirect-DMA gather of the 256 matching center rows, fused
(x-c)^2 row-sum on the vector engine, per-row clip, cross-partition sum via
a [128,1]x[128,1] matmul against ones. Host sums the 8 per-core partials
and divides by the global batch.
"""

import numpy as np

NUM_CLASSES = 100000
FEAT_DIM = 256
BATCH = 2048
N_CORES = 8
P = 128
B_LOCAL = BATCH // N_CORES          # 256 rows per core
N_TILES = B_LOCAL // P              # 2 tiles of 128 rows

_CACHE: dict = {}


def _build_bass():
    import concourse.bass as bass
    import concourse.bacc as bacc
    import concourse.mybir as mybir
    from concourse.tile import TileContext

    f32 = mybir.dt.float32
    i32 = mybir.dt.int32

    nc = bacc.Bacc(None, target_bir_lowering=False)
    x_in = nc.dram_tensor("x", [B_LOCAL, FEAT_DIM], f32, kind="ExternalInput")
    lab_in = nc.dram_tensor("labels", [B_LOCAL, 1], i32, kind="ExternalInput")
    cen_in = nc.dram_tensor(
        "centers", [NUM_CLASSES, FEAT_DIM], f32, kind="ExternalInput"
    )
    out = nc.dram_tensor("partial", [1, 1], f32, kind="ExternalOutput")

    # View the 256 local rows as [128 partitions, 2 tiles, 256 feat]:
    # element [p, t, d] = row t*128+p.
    x_v = x_in.rearrange("(t p) d -> p t d", p=P)
    lab_v = lab_in.rearrange("(t p) 1 -> p t", p=P)

    with TileContext(nc) as tc:
        with (
            tc.tile_pool(name="sbuf", bufs=1) as pool,
            tc.tile_pool(name="psum", bufs=1, space="PSUM") as psum_pool,
        ):
            ones = pool.tile([P, 1], f32)
            nc.vector.memset(ones[:], 1.0)

            lab_t = pool.tile([P, N_TILES], i32)
            nc.sync.dma_start(out=lab_t[:], in_=lab_v)

            x_t = pool.tile([P, N_TILES, FEAT_DIM], f32)
            nc.sync.dma_start(out=x_t[:], in_=x_v)

            c_t = pool.tile([P, N_TILES, FEAT_DIM], f32)
            diff = pool.tile([P, N_TILES, FEAT_DIM], f32)
            sq = pool.tile([P, N_TILES, FEAT_DIM], f32)
            dist = pool.tile([P, N_TILES], f32)
            for t in range(N_TILES):
                nc.gpsimd.indirect_dma_start(
                    out=c_t[:, t, :],
                    out_offset=None,
                    in_=cen_in[:],
                    in_offset=bass.IndirectOffsetOnAxis(
                        ap=lab_t[:, t : t + 1], axis=0
                    ),
                )
            for t in range(N_TILES):
                nc.vector.tensor_sub(
                    diff[:, t, :], x_t[:, t, :], c_t[:, t, :]
                )
                nc.scalar.activation(
                    out=sq[:, t, :],
                    in_=diff[:, t, :],
                    func=mybir.ActivationFunctionType.Square,
                    accum_out=dist[:, t : t + 1],
                )

            dist_c = pool.tile([P, N_TILES], f32)
            nc.vector.tensor_scalar(
                out=dist_c[:],
                in0=dist[:],
                scalar1=1e-12,
                scalar2=1e12,
                op0=mybir.AluOpType.max,
                op1=mybir.AluOpType.min,
            )

            rowsum = pool.tile([P, 1], f32)
            nc.vector.reduce_sum(
                out=rowsum[:], in_=dist_c[:], axis=mybir.AxisListType.X
            )

            ps = psum_pool.tile([1, 1], f32, space="PSUM")
            nc.tensor.matmul(
                out=ps[:], lhsT=rowsum[:], rhs=ones[:], start=True, stop=True
            )

            res = pool.tile([1, 1], f32)
            nc.vector.tensor_copy(res[:], ps[:])
            nc.sync.dma_start(out=out[:, :], in_=res[:])

    nc.finalize()
    return nc


def _get_bass():
    if "nc" not in _CACHE:
        _CACHE["nc"] = _build_bass()
    return _CACHE["nc"]


def _run(in_maps, trace=False, **kwargs):
    from concourse.bass_utils import run_bass_kernel_spmd

    nc = _get_bass()
    return run_bass_kernel_spmd(
        nc, in_maps, core_ids=list(range(N_CORES)), trace=trace, **kwargs
    )


def _make_in_maps(x, labels, centers):
    x = np.ascontiguousarray(np.asarray(x), dtype=np.float32)
    labels = np.asarray(labels).astype(np.int32).reshape(BATCH, 1)
    centers = np.ascontiguousarray(np.asarray(centers), dtype=np.float32)
    in_maps = []
    for c in range(N_CORES):
        lo, hi = c * B_LOCAL, (c + 1) * B_LOCAL
        in_maps.append(
            {
                "x": np.ascontiguousarray(x[lo:hi]),
                "labels": np.ascontiguousarray(labels[lo:hi]),
                "centers": centers,
            }
        )
    return in_maps


def kernel(x, labels, centers):
    in_maps = _make_in_maps(x, labels, centers)
    res = _run(in_maps)
    total = sum(float(r["partial"][0, 0]) for r in res.results)
    return np.float32(total / BATCH)


# revision 12
# speedup vs baseline: 1.0902x; 1.0313x over previous
"""CenterLoss kernel for Trainium2, data-parallel over batch across 8 cores.

loss = mean_b clip(||x_b - centers[labels_b]||^2, 1e-12, 1e12)

Per core: 256 rows of x/labels (batch shard), full replicated centers table.
On device (raw Bass, manual semaphores — Tile's scheduler adds ~8us of
barrier/ordering overhead this tiny kernel doesn't need):
  - indirect-DMA gather of the 256 matching center rows (2x128),
  - diff = x - c on the vector engine,
  - square + row-sum via scalar-engine activation accumulate,
  - per-row clip to [1e-12, 1e12], pair-sum, then a [128,1]x[128,1] matmul
    against ones to reduce across partitions -> one f32 partial per core.
Host sums the 8 per-core partials and divides by the global batch.
"""

import numpy as np

NUM_CLASSES = 100000
FEAT_DIM = 256
BATCH = 2048
N_CORES = 8
P = 128
B_LOCAL = BATCH // N_CORES          # 256 rows per core
N_TILES = B_LOCAL // P              # 2 tiles of 128 rows

_CACHE: dict = {}


def _build_bass():
    import concourse.bacc as bacc
    import concourse.bass as bass
    import concourse.mybir as mybir

    f32 = mybir.dt.float32
    i32 = mybir.dt.int32
    Alu = mybir.AluOpType

    nc = bacc.Bacc(None, target_bir_lowering=False)
    x_in = nc.dram_tensor("x", [B_LOCAL, FEAT_DIM], f32, kind="ExternalInput")
    lab_in = nc.dram_tensor("labels", [B_LOCAL, 1], i32, kind="ExternalInput")
    cen_in = nc.dram_tensor(
        "centers", [NUM_CLASSES, FEAT_DIM], f32, kind="ExternalInput"
    )
    out = nc.dram_tensor("partial", [1, 1], f32, kind="ExternalOutput")

    # View the 256 local rows as [128 partitions, 2 tiles, 256 feat]:
    # element [p, t, d] = row 2p+t, so each partition reads a contiguous
    # 2-row run of x and a contiguous label pair. The (p,t)<->row mapping
    # is a permutation of the batch; the final sum doesn't care.
    x_v = x_in.rearrange("(p t) d -> p t d", t=N_TILES)
    lab_v = lab_in.rearrange("(p t) 1 -> p t", t=N_TILES)

    with (
        nc.sbuf_tensor([P, N_TILES], i32) as lab_t,
        nc.sbuf_tensor([P, N_TILES, FEAT_DIM], f32) as x_t,
        nc.sbuf_tensor([P, N_TILES, FEAT_DIM], f32) as c_t,
        nc.sbuf_tensor([P, N_TILES, FEAT_DIM], f32) as diff,
        nc.sbuf_tensor([P, N_TILES, FEAT_DIM], f32) as sq,
        nc.sbuf_tensor([P, N_TILES], f32) as dist,
        nc.sbuf_tensor([P, N_TILES], f32) as dist_c,
        nc.sbuf_tensor([P, 1], f32) as rowsum,
        nc.sbuf_tensor([P, 1], f32) as ones,
        nc.sbuf_tensor([1, 1], f32) as res,
        nc.psum_tensor([1, 1], f32) as ps,
        nc.semaphore() as dS,   # input DMAs (labels, x)
        nc.semaphore() as gS,   # gather DMAs
        nc.semaphore() as vS,   # vector milestones
        nc.semaphore() as sS,   # scalar milestones
        nc.semaphore() as tS,   # matmul done
        nc.semaphore() as oS,   # ones ready
    ):
        # input DMAs on the HW DGE
        nc.sync.dma_start(lab_t[:, :], lab_v).then_inc(dS, 16)
        nc.sync.dma_start(x_t[:], x_v).then_inc(dS, 16)

        # ones for the partition-reduce matmul; no upstream deps
        nc.vector.memset(ones[:], 1.0).then_inc(oS, 1)

        # gather the matching center rows, one row per partition per op
        # (wait for both input DMAs — HWDGE completion order isn't guaranteed)
        nc.gpsimd.wait_ge(dS, 32)
        for t in range(N_TILES):
            nc.gpsimd.indirect_dma_start(
                out=c_t[:, t, :],
                out_offset=None,
                in_=cen_in[:],
                in_offset=bass.IndirectOffsetOnAxis(
                    ap=lab_t[:, t : t + 1], axis=0
                ),
            ).then_inc(gS, 16)

        # diff = x - c, per tile so t=0 overlaps the t=1 gather
        nc.vector.wait_ge(dS, 32)
        nc.vector.wait_ge(gS, 16)
        nc.vector.tensor_sub(diff[:, 0, :], x_t[:, 0, :], c_t[:, 0, :]).then_inc(
            vS, 1
        )
        nc.vector.wait_ge(gS, 32)
        nc.vector.tensor_sub(diff[:, 1, :], x_t[:, 1, :], c_t[:, 1, :]).then_inc(
            vS, 1
        )

        # dist[:, t] = sum_d diff^2 via activation accumulate
        nc.scalar.wait_ge(vS, 1)
        nc.scalar.activation(
            out=sq[:, 0, :],
            in_=diff[:, 0, :],
            func=mybir.ActivationFunctionType.Square,
            accum_out=dist[:, 0:1],
        ).then_inc(sS, 1)
        nc.scalar.wait_ge(vS, 2)
        nc.scalar.activation(
            out=sq[:, 1, :],
            in_=diff[:, 1, :],
            func=mybir.ActivationFunctionType.Square,
            accum_out=dist[:, 1:2],
        ).then_inc(sS, 1)

        # per-row clip, then sum the two per-partition rows. The sem
        # round-trip between clip and reduce guards the same-engine RAW on
        # dist_c — back-to-back DVE ops don't forward through the pipeline.
        nc.vector.wait_ge(sS, 2)
        nc.vector.tensor_scalar(
            out=dist_c[:],
            in0=dist[:],
            scalar1=1e-12,
            scalar2=1e12,
            op0=Alu.max,
            op1=Alu.min,
        ).then_inc(vS, 1)
        nc.vector.wait_ge(vS, 3)
        nc.vector.reduce_sum(
            out=rowsum[:], in_=dist_c[:], axis=mybir.AxisListType.X
        ).then_inc(vS, 1)

        # cross-partition sum: [1,128] @ [128,1] matmul against ones
        nc.tensor.wait_ge(vS, 4)
        nc.tensor.wait_ge(oS, 1)
        nc.tensor.matmul(
            out=ps[:], lhsT=rowsum[:], rhs=ones[:], start=True, stop=True
        ).then_inc(tS, 1)

        nc.vector.wait_ge(tS, 1)
        nc.vector.tensor_copy(res[:], ps[:]).then_inc(vS, 1)

        nc.sync.wait_ge(vS, 5)
        nc.sync.dma_start(out[:, :], res[:]).then_inc(dS, 16)

    nc.finalize()
    return nc


def _get_bass():
    if "nc" not in _CACHE:
        _CACHE["nc"] = _build_bass()
    return _CACHE["nc"]


def _run(in_maps, trace=False, **kwargs):
    from concourse.bass_utils import run_bass_kernel_spmd

    nc = _get_bass()
    return run_bass_kernel_spmd(
        nc, in_maps, core_ids=list(range(N_CORES)), trace=trace, **kwargs
    )


def _make_in_maps(x, labels, centers):
    x = np.ascontiguousarray(np.asarray(x), dtype=np.float32)
    labels = np.asarray(labels).astype(np.int32).reshape(BATCH, 1)
    centers = np.ascontiguousarray(np.asarray(centers), dtype=np.float32)
    in_maps = []
    for c in range(N_CORES):
        lo, hi = c * B_LOCAL, (c + 1) * B_LOCAL
        in_maps.append(
            {
                "x": np.ascontiguousarray(x[lo:hi]),
                "labels": np.ascontiguousarray(labels[lo:hi]),
                "centers": centers,
            }
        )
    return in_maps


def kernel(x, labels, centers):
    in_maps = _make_in_maps(x, labels, centers)
    res = _run(in_maps)
    total = sum(float(r["partial"][0, 0]) for r in res.results)
    return np.array(total / BATCH, dtype=np.float32)


# revision 15
# speedup vs baseline: 1.1719x; 1.0749x over previous
"""CenterLoss kernel for Trainium2, data-parallel over batch across 8 cores.

loss = mean_b clip(||x_b - centers[labels_b]||^2, 1e-12, 1e12)

Per core: 256 rows of x/labels (batch shard), full replicated centers table.
On device (raw Bass, manual semaphores — Tile's scheduler adds ~8us of
barrier/ordering overhead this tiny kernel doesn't need):
  - indirect-DMA gather of the 256 matching center rows (2x128),
  - diff = x - c on the vector engine,
  - square + row-sum via scalar-engine activation accumulate,
  - per-row clip to [1e-12, 1e12], pair-sum, then a [128,1]x[128,1] matmul
    against ones to reduce across partitions -> one f32 partial per core.
Host sums the 8 per-core partials and divides by the global batch.
"""

import numpy as np

NUM_CLASSES = 100000
FEAT_DIM = 256
BATCH = 2048
N_CORES = 8
P = 128
B_LOCAL = BATCH // N_CORES          # 256 rows per core
N_TILES = B_LOCAL // P              # 2 tiles of 128 rows

_CACHE: dict = {}


def _build_bass():
    import concourse.bacc as bacc
    import concourse.bass as bass
    import concourse.mybir as mybir

    f32 = mybir.dt.float32
    i32 = mybir.dt.int32
    Alu = mybir.AluOpType

    nc = bacc.Bacc(None, target_bir_lowering=False)
    x_in = nc.dram_tensor("x", [B_LOCAL, FEAT_DIM], f32, kind="ExternalInput")
    lab_in = nc.dram_tensor("labels", [B_LOCAL, 1], i32, kind="ExternalInput")
    cen_in = nc.dram_tensor(
        "centers", [NUM_CLASSES, FEAT_DIM], f32, kind="ExternalInput"
    )
    out = nc.dram_tensor("partial", [1, 1], f32, kind="ExternalOutput")

    # View the 256 local rows as [128 partitions, 2 tiles, 256 feat]:
    # element [p, t, d] = row 2p+t, so each partition reads a contiguous
    # 2-row run of x and a contiguous label pair. The (p,t)<->row mapping
    # is a permutation of the batch; the final sum doesn't care.
    x_v = x_in.rearrange("(p t) d -> p t d", t=N_TILES)
    lab_v = lab_in.rearrange("(p t) 1 -> p t", t=N_TILES)

    with (
        nc.sbuf_tensor([P, N_TILES], i32) as lab_t,
        nc.sbuf_tensor([P, N_TILES, FEAT_DIM], f32) as x_t,
        nc.sbuf_tensor([P, N_TILES, FEAT_DIM], f32) as c_t,
        nc.sbuf_tensor([P, N_TILES, FEAT_DIM], f32) as diff,
        nc.sbuf_tensor([P, N_TILES, FEAT_DIM], f32) as sq,
        nc.sbuf_tensor([P, N_TILES], f32) as dist,
        nc.sbuf_tensor([P, N_TILES], f32) as dist_c,
        nc.sbuf_tensor([P, 1], f32) as rowsum,
        nc.sbuf_tensor([P, 1], f32) as ones,
        nc.sbuf_tensor([1, 1], f32) as res,
        nc.psum_tensor([1, 1], f32) as ps,
        nc.semaphore() as lS,   # labels DMA
        nc.semaphore() as dS,   # x DMA + output DMA
        nc.semaphore() as gS,   # gather DMAs
        nc.semaphore() as vS,   # vector milestones
        nc.semaphore() as sS,   # scalar milestones
        nc.semaphore() as tS,   # matmul done
        nc.semaphore() as oS,   # ones ready
    ):
        # input DMAs on the HW DGE; labels first — they gate the gathers
        nc.sync.dma_start(lab_t[:, :], lab_v).then_inc(lS, 16)
        nc.sync.dma_start(x_t[:], x_v).then_inc(dS, 16)

        # ones for the partition-reduce matmul; no upstream deps
        nc.vector.memset(ones[:], 1.0).then_inc(oS, 1)

        # gather the matching center rows, one row per partition per op
        nc.gpsimd.wait_ge(lS, 16)
        for t in range(N_TILES):
            nc.gpsimd.indirect_dma_start(
                out=c_t[:, t, :],
                out_offset=None,
                in_=cen_in[:],
                in_offset=bass.IndirectOffsetOnAxis(
                    ap=lab_t[:, t : t + 1], axis=0
                ),
            ).then_inc(gS, 16)

        # diff = x - c, per tile so t=0 overlaps the t=1 gather
        nc.vector.wait_ge(dS, 16)
        nc.vector.wait_ge(gS, 16)
        nc.vector.tensor_sub(diff[:, 0, :], x_t[:, 0, :], c_t[:, 0, :]).then_inc(
            vS, 1
        )
        nc.vector.wait_ge(gS, 32)
        nc.vector.tensor_sub(diff[:, 1, :], x_t[:, 1, :], c_t[:, 1, :]).then_inc(
            vS, 1
        )

        # dist[:, t] = sum_d diff^2 via activation accumulate
        nc.scalar.wait_ge(vS, 1)
        nc.scalar.activation(
            out=sq[:, 0, :],
            in_=diff[:, 0, :],
            func=mybir.ActivationFunctionType.Square,
            accum_out=dist[:, 0:1],
        ).then_inc(sS, 1)
        nc.scalar.wait_ge(vS, 2)
        nc.scalar.activation(
            out=sq[:, 1, :],
            in_=diff[:, 1, :],
            func=mybir.ActivationFunctionType.Square,
            accum_out=dist[:, 1:2],
        ).then_inc(sS, 1)

        # per-row clip, then sum the two per-partition rows. The sem
        # round-trip between clip and reduce guards the same-engine RAW on
        # dist_c — back-to-back DVE ops don't forward through the pipeline.
        nc.vector.wait_ge(sS, 2)
        nc.vector.tensor_scalar(
            out=dist_c[:],
            in0=dist[:],
            scalar1=1e-12,
            scalar2=1e12,
            op0=Alu.max,
            op1=Alu.min,
        ).then_inc(vS, 1)
        nc.vector.wait_ge(vS, 3)
        nc.vector.reduce_sum(
            out=rowsum[:], in_=dist_c[:], axis=mybir.AxisListType.X
        ).then_inc(vS, 1)

        # cross-partition sum: [1,128] @ [128,1] matmul against ones
        nc.tensor.wait_ge(vS, 4)
        nc.tensor.wait_ge(oS, 1)
        nc.tensor.matmul(
            out=ps[:], lhsT=rowsum[:], rhs=ones[:], start=True, stop=True
        ).then_inc(tS, 1)

        nc.vector.wait_ge(tS, 1)
        nc.vector.tensor_copy(res[:], ps[:]).then_inc(vS, 1)

        nc.sync.wait_ge(vS, 5)
        nc.sync.dma_start(out[:, :], res[:]).then_inc(dS, 16)

    nc.finalize()
    return nc


def _get_bass():
    if "nc" not in _CACHE:
        _CACHE["nc"] = _build_bass()
    return _CACHE["nc"]


def _run(in_maps, trace=False, **kwargs):
    from concourse.bass_utils import run_bass_kernel_spmd

    nc = _get_bass()
    return run_bass_kernel_spmd(
        nc, in_maps, core_ids=list(range(N_CORES)), trace=trace, **kwargs
    )


def _make_in_maps(x, labels, centers):
    x = np.ascontiguousarray(np.asarray(x), dtype=np.float32)
    labels = np.asarray(labels).astype(np.int32).reshape(BATCH, 1)
    centers = np.ascontiguousarray(np.asarray(centers), dtype=np.float32)
    in_maps = []
    for c in range(N_CORES):
        lo, hi = c * B_LOCAL, (c + 1) * B_LOCAL
        in_maps.append(
            {
                "x": np.ascontiguousarray(x[lo:hi]),
                "labels": np.ascontiguousarray(labels[lo:hi]),
                "centers": centers,
            }
        )
    return in_maps


def kernel(x, labels, centers):
    in_maps = _make_in_maps(x, labels, centers)
    res = _run(in_maps)
    total = sum(float(r["partial"][0, 0]) for r in res.results)
    return np.array(total / BATCH, dtype=np.float32)
